# revision 1
# baseline (speedup 1.0000x reference)
"""Trainium2 Bass kernel for nn_ArcDecoderLayer (sparse_attention).

Self-contained: takes FULL unsharded inputs, shards across 8 NeuronCores
(head-parallel attention + FF-parallel MLP, AllGather-only collectives),
returns the FULL output.

Layout strategy: everything on device lives transposed [feature, seq]
so matmuls chain without any on-device transposes (host pre-transposes).
LayerNorm1 is folded into the QKV projection epilogues; LN2 is applied
directly. All matmul compute in bf16 with f32 PSUM accumulation.
"""

import sys
import types

sys.path.insert(0, "/opt/trn_rl_repo")

# ---- shim antenv.axon_hooks so trace=True profiling works in this image ----
if "antenv.axon_hooks" not in sys.modules:
    _hook_mod = types.ModuleType("antenv.axon_hooks")
    _hook_state = {"hook": None}

    def _set_hook(h):
        _hook_state["hook"] = h

    def _get_hook():
        return _hook_state["hook"]

    _hook_mod.set_axon_ntff_profile_hook = _set_hook
    _hook_mod.get_axon_ntff_profile_hook = _get_hook
    sys.modules["antenv.axon_hooks"] = _hook_mod
    try:
        import antenv

        antenv.axon_hooks = _hook_mod
        from trn_agent_boot.trn_boot import _ntff_profile_via_ctypes

        _set_hook(_ntff_profile_via_ctypes("/opt/axon/libaxon_pjrt.so"))
    except Exception:
        pass

import numpy as np
import ml_dtypes

import concourse.bass as bass
import concourse.mybir as mybir
import concourse.tile as tile
from concourse import library_config
from concourse.vector_clock import ScopedClock

BF16 = ml_dtypes.bfloat16

N_CORES = 8
D = 2048
FF = 8192
H = 32
DH = 64
RD = 16
EPS = 1e-5
BASE = 10000.0

J = D // N_CORES        # 256 head-dims per core (4 heads)
FFL = FF // N_CORES     # 1024 ff dims per core
KC = D // 128           # 16 contraction chunks
NBLK = 512              # lq block width
MD = J // 128           # 2 output Mtiles per core for o/down


WAIT_LIMITS = {"InstNoOp": 1, "InstDrain": 1, "InstEventSemaphore": 1}
DEFAULT_WAIT_LIMIT = 1


class PatchedTC(tile.TileContext):
    """TileContext patched for this walrus build, which rejects instructions
    carrying more than a couple of sync wait commands: excess waits are
    split onto injected same-engine nops just before the instruction."""

    _wsplit_n = 0

    def _split_excess_waits(self, ordered):
        for bb, insts in ordered.items():
            out = []
            for inst in insts:
                si = inst.sync_info
                waits = list(si.on_wait) if si and si.on_wait else []
                lim = WAIT_LIMITS.get(type(inst).__name__,
                                      DEFAULT_WAIT_LIMIT)
                if len(waits) > lim:
                    for w in waits[:-lim]:
                        nop = mybir.InstNoOp(
                            name=f"I-wsplit-{PatchedTC._wsplit_n}",
                            ins=[], outs=[], engine=inst.engine,
                            nofuse=True)
                        PatchedTC._wsplit_n += 1
                        nop.sync_info = mybir.SyncInfo(
                            on_wait=[w], on_update=[])
                        out.append(nop)
                    inst.sync_info = mybir.SyncInfo(
                        on_wait=waits[-lim:],
                        on_update=list(si.on_update or []))
                out.append(inst)
            ordered[bb] = out

    def _lower_ordered_insts(self, ordered):
        self._split_excess_waits(ordered)
        return super()._lower_ordered_insts(ordered)

    def _drain_and_barrier(self, tick_clock, wait_clock):
        nc = self.nc
        probe = nc.sync.nop(nofuse=True, hint="tail_wait_probe")
        wait_clock.add_sem_waits(
            probe.ins, ScopedClock({None: tick_clock.global_clock})
        )
        waits = list(probe.ins.sync_info.on_wait or [])
        probe.ins.sync_info.on_wait = waits[:1]
        for i in range(1, len(waits)):
            n = nc.sync.nop(nofuse=True, hint=f"tail_wait_{i}")
            n.ins.sync_info = mybir.SyncInfo(on_wait=[waits[i]], on_update=[])
        nc.sync.drain()
        nc.all_engine_barrier()
        assert self.sems is not None
        popped = nc._tile_sem_poison_stack.pop()
        assert popped is self._sem_poison
        nc.clear_and_free_semaphores(list(self.sems.allocated().values()))
        nc.all_engine_barrier()


def build_graph(S):
    """Build the SPMD 8-core graph for sequence length S (multiple of 512)."""
    dt = mybir.dt
    f32, bf16 = dt.float32, dt.bfloat16
    AF = mybir.ActivationFunctionType
    Alu = mybir.AluOpType
    NB = S // NBLK          # lq blocks
    LT = S // 128           # 128-wide l tiles per part
    S2 = 2 * S

    nc = bass.Bass()
    P = nc.declare_dram_parameter

    xm_e = P("xm", [128, KC, S], bf16, isOutput=False)
    xh_e = P("xh", [128, KC, S], bf16, isOutput=False)
    xhres_e = P("xh_res", [128, MD, S], f32, isOutput=False)
    wq_e = P("wq", [128, KC, J], bf16, isOutput=False)
    wk_e = P("wk", [128, KC, J], bf16, isOutput=False)
    wv_e = P("wv", [128, KC, J], bf16, isOutput=False)
    wo_e = P("wo", [128, KC, J], bf16, isOutput=False)
    wg_e = P("wg", [128, KC, FFL], bf16, isOutput=False)
    wu_e = P("wu", [128, KC, FFL], bf16, isOutput=False)
    wd_e = P("wd", [128, FF // 128, J], bf16, isOutput=False)
    # column (per-partition) weight rowsums + biases for q/k/vTh epilogues
    wsq_e = P("wsq", [128, 2], f32, isOutput=False)
    wsk_e = P("wsk", [128, 2], f32, isOutput=False)
    wsvc_e = P("wsvc", [128, 2], f32, isOutput=False)   # for vT_h epilogue
    bq_e = P("bq", [128, 2], f32, isOutput=False)
    bk_e = P("bk", [128, 2], f32, isOutput=False)
    bvc_e = P("bvc", [128, 2], f32, isOutput=False)
    # row layouts for v_mem epilogue
    wsv_e = P("wsv_row", [1, J], f32, isOutput=False)
    bv_e = P("bv_row", [1, J], f32, isOutput=False)
    bg_e = P("bg", [128, FFL // 128], f32, isOutput=False)
    bu_e = P("bu", [128, FFL // 128], f32, isOutput=False)
    ropec_e = P("rope_cos", [128, S2], bf16, isOutput=False)
    ropes_e = P("rope_sinsg", [128, S2], bf16, isOutput=False)
    masks_e = P("masks", [128, 4, NBLK], bf16, isOutput=False)
    out_e = P("out", [MD, 128, S], f32, isOutput=True)

    rg = [list(range(N_CORES))]

    with PatchedTC(nc) as tc:
        with (
            tc.tile_pool(name="const", bufs=1) as constp,
            tc.tile_pool(name="dram", bufs=1, space="DRAM") as dramp,
            tc.tile_pool(name="dsh", bufs=1, space="DRAM") as dshp,
        ):
            kqvp = tc.alloc_tile_pool(name="kqv", bufs=1)
            statkp = tc.alloc_tile_pool(name="statk", bufs=1)
            masks_t = constp.tile([128, 4, NBLK], bf16)
            nc.sync.dma_start(masks_t[:], masks_e[:])
            ones_c = constp.tile([128, 1], bf16)
            nc.vector.memset(ones_c[:], 1.0)
            ones128 = constp.tile([128, 128], bf16)
            nc.vector.memset(ones128[:], 1.0)
            eps_c = constp.tile([128, 1], f32)
            nc.vector.memset(eps_c[:], EPS)
            onesf = constp.tile([1, 128], f32)
            nc.vector.memset(onesf[:], 1.0)

            def bcast_rows(dst, src_row, width, pspool, ones_row):
                """dst[0:128, :width] = src_row[0, :width] via K=1 matmuls
                (partition_broadcast is not encodable by this walrus)."""
                for i in range(0, width, NBLK):
                    w = min(NBLK, width - i)
                    ps = pspool.tile([128, NBLK], f32, name="bc_ps",
                                     tag="bc_ps", bufs=1)
                    nc.tensor.matmul(ps[:, :w], ones_row[0:1, :],
                                     src_row[0:1, i:i + w],
                                     start=True, stop=True)
                    nc.vector.tensor_copy(dst[:, i:i + w], ps[:, :w])
            wsvb = constp.tile([128, J], f32)
            wsv_row = constp.tile([1, J], f32)
            nc.sync.dma_start(wsv_row[:], wsv_e[:])
            bvb = constp.tile([128, J], f32)
            bv_row = constp.tile([1, J], f32)
            nc.sync.dma_start(bv_row[:], bv_e[:])
            wsq_t = constp.tile([128, 2], f32)
            nc.sync.dma_start(wsq_t[:], wsq_e[:])
            wsk_t = constp.tile([128, 2], f32)
            nc.sync.dma_start(wsk_t[:], wsk_e[:])
            wsvc_t = constp.tile([128, 2], f32)
            nc.sync.dma_start(wsvc_t[:], wsvc_e[:])
            bq_t = constp.tile([128, 2], f32)
            nc.sync.dma_start(bq_t[:], bq_e[:])
            bk_t = constp.tile([128, 2], f32)
            nc.sync.dma_start(bk_t[:], bk_e[:])
            bvc_t = constp.tile([128, 2], f32)
            nc.sync.dma_start(bvc_t[:], bvc_e[:])
            bg_t = constp.tile([128, FFL // 128], f32)
            nc.sync.dma_start(bg_t[:], bg_e[:])
            bu_t = constp.tile([128, FFL // 128], f32)
            nc.sync.dma_start(bu_t[:], bu_e[:])

            # persistent QKV outputs
            kT = [kqvp.tile([128, S2], bf16, name=f"kT{m}") for m in range(2)]
            qT = [kqvp.tile([128, S], bf16, name=f"qT{m}") for m in range(2)]
            vTh = [kqvp.tile([128, S], bf16, name=f"vTh{m}") for m in range(2)]
            v_mem = kqvp.tile([128, LT, J], bf16)

            # v_mem epilogue needs column-layout stats of the mem part
            rstd_col_mem = statkp.tile([128, LT], f32)
            c_col_mem = statkp.tile([128, LT], f32)

            # ---------- LN1 stats + QKV -----------------------------------
            def ln_stats(xpart, sqp, psp, smallp, rowp, part_name):
                """Returns (rstd_col, c_col, rstd_b, c_b) for one x part.

                xpart: [128, KC, S] bf16. Stats are over the 128*KC feature
                dim per l column. col layout: [128, LT] (l = t*128 + p);
                row/bcast layout: [128, S] bf16 (broadcast along partitions).
                Sum is accumulated column-major (ones as rhs, sequential
                per-column groups); sumsq row-major (ones as lhsT) since
                interleaved PSUM accumulation groups in one bank are illegal.
                """
                sum_ps = psp.tile([128, LT], f32, name="sum_ps",
                                  tag="sum_ps")
                for lt in range(LT):
                    sl = slice(lt * 128, (lt + 1) * 128)
                    for kc in range(KC):
                        nc.tensor.matmul(
                            sum_ps[:, lt:lt + 1],
                            xpart[:, kc, sl], ones_c[:],
                            start=(kc == 0), stop=(kc == KC - 1))
                sumsq_row = rowp.tile([1, S], f32, name="sumsq_row",
                                      tag="strow")
                for nb in range(NB):
                    sq_ps = psp.tile([1, NBLK], f32, name="sq_ps",
                                     tag="sq_ps")
                    for kc in range(KC):
                        sq_t = sqp.tile([128, NBLK], bf16, name="sq_t")
                        nc.scalar.activation(
                            sq_t[:], xpart[:, kc, nb * NBLK:(nb + 1) * NBLK],
                            AF.Square)
                        nc.tensor.matmul(
                            sq_ps[:], ones_c[:], sq_t[:],
                            start=(kc == 0), stop=(kc == KC - 1))
                    nc.vector.tensor_copy(
                        sumsq_row[:, nb * NBLK:(nb + 1) * NBLK], sq_ps[:])
                # sumsq row -> col via DRAM
                drq = dramp.tile([S], f32, name=f"st_sq_{part_name}")
                nc.sync.dma_start(
                    drq[:].rearrange("(o a) -> o a", o=1), sumsq_row[:])
                sumsq_col = smallp.tile([128, LT], f32, name="sumsq_col")
                nc.sync.dma_start(
                    sumsq_col[:], drq[:].rearrange("(t p) -> p t", p=128))
                mean_c = smallp.tile([128, LT], f32, name="mean_c")
                nc.vector.tensor_scalar_mul(mean_c[:], sum_ps[:], 1.0 / D)
                ex2_c = smallp.tile([128, LT], f32, name="ex2_c")
                nc.vector.tensor_scalar_mul(ex2_c[:], sumsq_col[:], 1.0 / D)
                m2_c = smallp.tile([128, LT], f32, name="m2_c")
                nc.vector.tensor_mul(m2_c[:], mean_c[:], mean_c[:])
                var_c = smallp.tile([128, LT], f32, name="var_c")
                nc.vector.tensor_sub(var_c[:], ex2_c[:], m2_c[:])
                sd_c = smallp.tile([128, LT], f32, name="sd_c")
                nc.scalar.activation(sd_c[:], var_c[:], AF.Sqrt, bias=eps_c[:])
                rstd_c = smallp.tile([128, LT], f32, name="rstd_c")
                nc.vector.reciprocal(rstd_c[:], sd_c[:])
                c_c = smallp.tile([128, LT], f32, name="c_c")
                nc.vector.tensor_mul(c_c[:], mean_c[:], rstd_c[:])
                # col -> row roundtrip through DRAM, then partition-broadcast
                outs = []
                for nm, col in (("rstd", rstd_c), ("c", c_c)):
                    dr = dramp.tile([S], f32, name=f"st_{nm}_{part_name}")
                    nc.sync.dma_start(
                        dr[:].rearrange("(t p) -> p t", p=128), col[:])
                    row = rowp.tile([1, S], f32, name=f"row_{nm}",
                                    tag="strow")
                    nc.sync.dma_start(
                        row[:], dr[:].rearrange("(o a) -> o a", o=1))
                    row16 = rowp.tile([1, S], bf16, name=f"row16_{nm}",
                                      tag="strow16")
                    nc.vector.tensor_copy(row16[:], row[:])
                    bcast = rowp.tile([128, S], bf16, name=f"bc_{nm}")
                    bcast_rows(bcast, row16, S, psp, ones128)
                    outs.append(bcast)
                return rstd_c, c_c, outs[0], outs[1]

            def cw_tile(pool, c_b, ws_t, b_t, m, name):
                cw = pool.tile([128, S], bf16, name=name)
                nc.vector.tensor_scalar(
                    out=cw[:], in0=c_b[:], scalar1=ws_t[:, m:m + 1],
                    scalar2=b_t[:, m:m + 1],
                    op0=Alu.mult, op1=Alu.subtract)
                return cw

            def proj_rows(wt, dst, dst_off, xpart, rstd_b, cws, psp, scrp):
                """q/k/vTh-style projection. Raw matmul results are copied
                to dst immediately (PE never stalls on LN stats); the LN
                epilogue (rstd mul, cw subtract) is applied in-place after
                stats are ready."""
                for m in range(2):
                    for nb in range(NB):
                        ps = psp.tile([128, NBLK], f32, name="proj_ps")
                        for kc in range(KC):
                            nc.tensor.matmul(
                                ps[:],
                                wt[:, kc, m * 128:(m + 1) * 128],
                                xpart[:, kc, nb * NBLK:(nb + 1) * NBLK],
                                start=(kc == 0), stop=(kc == KC - 1))
                        d = dst[m][:, dst_off + nb * NBLK:
                                   dst_off + (nb + 1) * NBLK]
                        nc.vector.tensor_copy(d, ps[:])
                for m in range(2):
                    for nb in range(NB):
                        sl = slice(nb * NBLK, (nb + 1) * NBLK)
                        d = dst[m][:, dst_off + nb * NBLK:
                                   dst_off + (nb + 1) * NBLK]
                        nc.vector.tensor_mul(d, d, rstd_b[:, sl])
                        nc.vector.tensor_sub(d, d, cws[m][:, sl])

            with (
                tc.tile_pool(name="wqkv", bufs=1) as wqkvp,
                tc.tile_pool(name="psq", bufs=2, space="PSUM") as psqp,
                tc.tile_pool(name="psst", bufs=1, space="PSUM") as psstp,
            ):
                wq_t = wqkvp.tile([128, KC, J], bf16)
                nc.sync.dma_start(wq_t[:], wq_e[:])
                wk_t = wqkvp.tile([128, KC, J], bf16)
                nc.sync.dma_start(wk_t[:], wk_e[:])
                wv_t = wqkvp.tile([128, KC, J], bf16)
                nc.sync.dma_start(wv_t[:], wv_e[:])

                # ----- phase 1a: memory part -----
                with (
                    tc.tile_pool(name="xm", bufs=1) as xmp,
                    tc.tile_pool(name="sqa", bufs=2) as sqap,
                    tc.tile_pool(name="sma", bufs=1) as smap,
                    tc.tile_pool(name="rowa", bufs=1) as rowap,
                ):
                    xm_t = xmp.tile([128, KC, S], bf16)
                    for kc in range(KC):
                        eng = nc.sync if kc % 2 == 0 else nc.scalar
                        eng.dma_start(xm_t[:, kc, :], xm_e[:, kc, :])
                    rs_c, c_c, rstd_bm, c_bm = ln_stats(
                        xm_t, sqap, psstp, smap, rowap, "mem")
                    nc.vector.tensor_copy(rstd_col_mem[:], rs_c[:])
                    nc.vector.tensor_copy(c_col_mem[:], c_c[:])
                    bcast_rows(wsvb, wsv_row, J, psqp, onesf)
                    bcast_rows(bvb, bv_row, J, psqp, onesf)
                    cwk_m = [cw_tile(rowap, c_bm, wsk_t, bk_t, m, f"cwkm{m}")
                             for m in range(2)]
                    proj_rows(wk_t, kT, 0, xm_t, rstd_bm, cwk_m, psqp, sqap)
                    # v_mem row-major: lhsT = xm l-tile, rhs = wv
                    for lt in range(LT):
                        ps = psqp.tile([128, J], f32, name="vm_ps",
                                       bufs=2)
                        for kc in range(KC):
                            nc.tensor.matmul(
                                ps[:],
                                xm_t[:, kc, lt * 128:(lt + 1) * 128],
                                wv_t[:, kc, :],
                                start=(kc == 0), stop=(kc == KC - 1))
                        nc.vector.tensor_copy(v_mem[:, lt, :], ps[:])
                    for lt in range(LT):
                        cwv = sqap.tile([128, J], f32, name="cwv")
                        nc.vector.tensor_scalar(
                            out=cwv[:], in0=wsvb[:],
                            scalar1=c_col_mem[:, lt:lt + 1], scalar2=None,
                            op0=Alu.mult)
                        nc.vector.tensor_scalar_mul(
                            v_mem[:, lt, :], v_mem[:, lt, :],
                            rstd_col_mem[:, lt:lt + 1])
                        nc.vector.tensor_sub(
                            v_mem[:, lt, :], v_mem[:, lt, :], cwv[:])
                        nc.vector.tensor_add(
                            v_mem[:, lt, :], v_mem[:, lt, :], bvb[:])

                # ----- phase 1b: hidden part -----
                with (
                    tc.tile_pool(name="xh", bufs=1) as xhp,
                    tc.tile_pool(name="sqb", bufs=2) as sqbp,
                    tc.tile_pool(name="smb", bufs=1) as smbp,
                    tc.tile_pool(name="rowb", bufs=1) as rowbp,
                ):
                    xh_t = xhp.tile([128, KC, S], bf16)
                    for kc in range(KC):
                        eng = nc.sync if kc % 2 == 0 else nc.scalar
                        eng.dma_start(xh_t[:, kc, :], xh_e[:, kc, :])
                    _, _, rstd_bh, c_bh = ln_stats(
                        xh_t, sqbp, psstp, smbp, rowbp, "hid")
                    cwq = [cw_tile(rowbp, c_bh, wsq_t, bq_t, m, f"cwq{m}")
                           for m in range(2)]
                    cwk_h = [cw_tile(rowbp, c_bh, wsk_t, bk_t, m, f"cwkh{m}")
                             for m in range(2)]
                    cwv_h = [cw_tile(rowbp, c_bh, wsvc_t, bvc_t, m, f"cwvh{m}")
                             for m in range(2)]
                    proj_rows(wq_t, qT, 0, xh_t, rstd_bh, cwq, psqp, sqbp)
                    proj_rows(wk_t, kT, S, xh_t, rstd_bh, cwk_h, psqp, sqbp)
                    proj_rows(wv_t, vTh, 0, xh_t, rstd_bh, cwv_h, psqp, sqbp)

                    # keep xh for the residual path: it is re-loaded later
                    # (xh pool closes here; phase 5 re-DMAs from DRAM)

            # ---------- phase 2: RoPE on qT / kT --------------------------
            with tc.tile_pool(name="rope", bufs=1) as ropep:
                cos_t = ropep.tile([128, S2], bf16)
                nc.sync.dma_start(cos_t[:], ropec_e[:])
                sin_t = ropep.tile([128, S2], bf16)
                nc.sync.dma_start(sin_t[:], ropes_e[:])

                def rope(dst_tiles, L):
                    for m in range(2):
                        t = dst_tiles[m]
                        for o in (0, 64):
                            sw = ropep.tile([128, S2], bf16, name="rope_sw",
                                            bufs=2)
                            # swap halves of the 16 rotary rows via DMA
                            nc.sync.dma_start(
                                sw[o:o + 8, :L], t[o + 8:o + 16, :L])
                            nc.sync.dma_start(
                                sw[o + 8:o + 16, :L], t[o:o + 8, :L])
                            tc_ = ropep.tile([128, S2], bf16, name="rope_tc",
                                             bufs=2)
                            nc.vector.tensor_mul(
                                tc_[o:o + 16, :L], t[o:o + 16, :L],
                                cos_t[o:o + 16, :L])
                            nc.vector.tensor_mul(
                                sw[o:o + 16, :L], sw[o:o + 16, :L],
                                sin_t[o:o + 16, :L])
                            nc.vector.tensor_add(
                                t[o:o + 16, :L], tc_[o:o + 16, :L],
                                sw[o:o + 16, :L])

                rope(qT, S)
                rope(kT, S2)

            # ---------- phase 3: attention --------------------------------
            GB = 2 if NB % 2 == 0 else 1   # blocks per AllGather chunk
            NG = NB // GB
            attn_bnc = [dramp.tile([J, GB * NBLK], bf16, name=f"attn_bnc{g}")
                        for g in range(NG)]
            attn_ag = [dshp.tile([D, GB * NBLK], bf16, name=f"attn_ag{g}",
                                 addr_space="Shared") for g in range(NG)]
            with (
                tc.tile_pool(name="maskp", bufs=1) as maskp,
                tc.tile_pool(name="attw", bufs=12) as attwp,
                tc.tile_pool(name="attt", bufs=4) as atttp,
                tc.tile_pool(name="attr", bufs=2) as attrp,
                tc.tile_pool(name="psS", bufs=3, space="PSUM") as psSp,
                tc.tile_pool(name="psA", bufs=1, space="PSUM") as psAp,
                tc.tile_pool(name="psB", bufs=1, space="PSUM") as psBp,
                tc.tile_pool(name="psD", bufs=1, space="PSUM") as psDp,
            ):
                masks_t = maskp.tile([128, 4, NBLK], bf16)
                nc.sync.dma_start(masks_t[:], masks_e[:])
                for b in range(NB):
                    bsl = slice(b * NBLK, (b + 1) * NBLK)
                    # den4/sf4 pack the 4 per-head [1, NBLK] rows into one
                    # bank each at partitions {0,32,64,96}
                    den4 = psDp.tile([128, NBLK], f32, name="den4")
                    sf4 = psDp.tile([128, NBLK], f32, name="sf4")
                    swf4 = attrp.tile([128, NBLK], f32, name="swf4")
                    dent4 = attrp.tile([128, NBLK], f32, name="dent4")
                    rcp4 = attrp.tile([128, NBLK], f32, name="rcp4")
                    swb4 = attrp.tile([128, NBLK], bf16, name="swb4")
                    rcpb4 = attrp.tile([128, NBLK], bf16, name="rcpb4")
                    for m in range(2):
                        ap_ps = psAp.tile([128, NBLK], f32, name="ap_ps")
                        for o in (0, 64):
                            hsl = slice(o, o + 64)
                            r = 32 * (2 * m + o // 64)
                            rsl = slice(r, r + 1)
                            for t in range(4 * b + 4):
                                s_ps = psSp.tile([128, NBLK], f32,
                                                 name="s_ps")
                                nc.tensor.matmul(
                                    s_ps[:],
                                    kT[m][hsl, t * 128:(t + 1) * 128],
                                    qT[m][hsl, bsl],
                                    start=True, stop=True,
                                    tile_position=(o, 0))
                                w_t = attwp.tile([128, NBLK], bf16,
                                                 name="w_t")
                                nc.scalar.activation(
                                    w_t[:], s_ps[:], AF.Exp, scale=0.125)
                                if t >= 4 * b:
                                    nc.vector.tensor_mul(
                                        w_t[:], w_t[:],
                                        masks_t[:, t - 4 * b, :])
                                nc.tensor.matmul(
                                    ap_ps[hsl, :],
                                    v_mem[:, t, m * 128 + o:
                                          m * 128 + o + 64],
                                    w_t[:],
                                    start=(t == 0), stop=(t == 4 * b + 3),
                                    tile_position=(0, o))
                                nc.tensor.matmul(
                                    den4[rsl, :], ones_c[:, 0:1], w_t[:],
                                    start=(t == 0), stop=(t == 4 * b + 3),
                                    tile_position=(0, r))
                            # self key: q . k_h elementwise + column sums
                            qk = atttp.tile([128, NBLK], bf16, name="qk")
                            nc.vector.tensor_mul(
                                qk[hsl, :], qT[m][hsl, bsl],
                                kT[m][hsl, S + b * NBLK:S + (b + 1) * NBLK])
                            nc.tensor.matmul(
                                sf4[rsl, :], ones_c[hsl, 0:1], qk[hsl, :],
                                start=True, stop=True,
                                tile_position=(o, r))
                            nc.scalar.activation(
                                swf4[rsl, :], sf4[rsl, :], AF.Exp,
                                scale=0.125)
                            nc.vector.tensor_copy(swb4[rsl, :], swf4[rsl, :])
                            nc.vector.tensor_add(
                                dent4[rsl, :], den4[rsl, :], swf4[rsl, :])
                            nc.vector.reciprocal(rcp4[rsl, :], dent4[rsl, :])
                            nc.vector.tensor_copy(rcpb4[rsl, :], rcp4[rsl, :])
                        # broadcast self_w and 1/den to each head's 64 rows
                        sb_ps = psBp.tile([128, NBLK], f32, name="sb_ps")
                        rb_ps = psBp.tile([128, NBLK], f32, name="rb_ps")
                        for o in (0, 64):
                            r = 32 * (2 * m + o // 64)
                            rsl = slice(r, r + 1)
                            nc.tensor.matmul(
                                sb_ps[o:o + 64, :], ones128[rsl, 0:64],
                                swb4[rsl, :], start=True, stop=True,
                                tile_position=(r, o))
                            nc.tensor.matmul(
                                rb_ps[o:o + 64, :], ones128[rsl, 0:64],
                                rcpb4[rsl, :], start=True, stop=True,
                                tile_position=(r, o))
                        # combine: (attn + self_w * vTh) / den
                        t0 = atttp.tile([128, NBLK], bf16, name="t0")
                        nc.vector.tensor_mul(t0[:], vTh[m][:, bsl], sb_ps[:])
                        t1 = atttp.tile([128, NBLK], bf16, name="t1")
                        nc.vector.tensor_add(t1[:], ap_ps[:], t0[:])
                        cmb = atttp.tile([128, NBLK], bf16, name="cmb")
                        nc.vector.tensor_mul(cmb[:], t1[:], rb_ps[:])
                        nc.scalar.dma_start(
                            attn_bnc[b // GB][m * 128:(m + 1) * 128,
                                              (b % GB) * NBLK:
                                              (b % GB + 1) * NBLK], cmb[:])
                    if b % GB == GB - 1:
                        g = b // GB
                        nc.gpsimd.collective_compute(
                            "AllGather", mybir.AluOpType.bypass,
                            replica_groups=rg,
                            ins=[attn_bnc[g].opt()], outs=[attn_ag[g].opt()])
            statkp.release()
            kqvp.release()

            # ---------- phase 4+5: o_proj, residual, LN2 ------------------
            o_bnc = [dramp.tile([J, GB * NBLK], bf16, name=f"o_bnc{g}")
                     for g in range(NG)]
            o_ag = [dshp.tile([D, GB * NBLK], bf16, name=f"o_ag{g}",
                              addr_space="Shared") for g in range(NG)]
            res_pool = tc.alloc_tile_pool(name="res", bufs=1)
            res_t = res_pool.tile([128, MD, S], bf16)
            h2_pool = tc.alloc_tile_pool(name="h2", bufs=1)
            h2_t = h2_pool.tile([128, KC, S], bf16)
            xhres_pool = tc.alloc_tile_pool(name="xhres", bufs=1)
            xhres_t = xhres_pool.tile([128, MD, S], f32)
            nc.sync.dma_start(xhres_t[:], xhres_e[:])
            with (
                tc.tile_pool(name="wo", bufs=1) as wop,
                tc.tile_pool(name="attf", bufs=1) as attfp,
                tc.tile_pool(name="xh2", bufs=1) as xh2p,
                tc.tile_pool(name="hblk", bufs=2) as hblkp,
                tc.tile_pool(name="sq5", bufs=2) as sq5p,
                tc.tile_pool(name="sm5", bufs=1) as sm5p,
                tc.tile_pool(name="psO", bufs=2, space="PSUM") as psOp,
                tc.tile_pool(name="psst5", bufs=2, space="PSUM") as psst5p,
            ):
                wo_t = wop.tile([128, KC, J], bf16)
                nc.sync.dma_start(wo_t[:], wo_e[:])
                for b in range(NB):
                    bsl = slice(b * NBLK, (b + 1) * NBLK)
                    csl = slice((b % GB) * NBLK, (b % GB + 1) * NBLK)
                    af = attfp.tile([128, KC, NBLK], bf16, name="af")
                    nc.sync.dma_start(
                        af[:], attn_ag[b // GB][:, csl].rearrange(
                            "(t p) s -> p t s", p=128))
                    for md in range(MD):
                        ps = psOp.tile([128, NBLK], f32, name="o_ps")
                        for kc in range(KC):
                            nc.tensor.matmul(
                                ps[:], wo_t[:, kc, md * 128:(md + 1) * 128],
                                af[:, kc, :],
                                start=(kc == 0), stop=(kc == KC - 1))
                        oc = attfp.tile([128, NBLK], bf16, name="oc")
                        nc.vector.tensor_copy(oc[:], ps[:])
                        nc.scalar.dma_start(
                            o_bnc[b // GB][md * 128:(md + 1) * 128, csl],
                            oc[:])
                        nc.vector.tensor_add(
                            res_t[:, md, bsl], xhres_t[:, md, bsl], ps[:])
                    if b % GB == GB - 1:
                        g = b // GB
                        nc.gpsimd.collective_compute(
                            "AllGather", mybir.AluOpType.bypass,
                            replica_groups=rg,
                            ins=[o_bnc[g].opt()], outs=[o_ag[g].opt()])
                # h = xh + o, LN2, h2 (per block; overlaps o_proj of later
                # groups via the Tile scheduler)
                for b in range(NB):
                    bsl = slice(b * NBLK, (b + 1) * NBLK)
                    csl = slice((b % GB) * NBLK, (b % GB + 1) * NBLK)
                    xhb = xh2p.tile([128, KC, NBLK], bf16, name="xhb")
                    nc.sync.dma_start(xhb[:], xh_e[:, :, bsl])
                    of = hblkp.tile([128, KC, NBLK], bf16, name="of",
                                     bufs=1)
                    nc.sync.dma_start(
                        of[:], o_ag[b // GB][:, csl].rearrange(
                            "(t p) s -> p t s", p=128))
                    h_t = hblkp.tile([128, KC, NBLK], bf16, name="h_t")
                    for kc in range(KC):
                        nc.vector.tensor_add(
                            h_t[:, kc, :], xhb[:, kc, :], of[:, kc, :])
                    # LN2 stats for this block (row-major [1, NBLK])
                    sum_ps = psst5p.tile([1, NBLK], f32, name="sum5_ps")
                    sq_ps = psst5p.tile([1, NBLK], f32, name="sq5_ps")
                    for kc in range(KC):
                        sq_t = sq5p.tile([128, NBLK], bf16, name="sq5_t")
                        nc.scalar.activation(sq_t[:], h_t[:, kc, :],
                                             AF.Square)
                        nc.tensor.matmul(
                            sum_ps[:], ones_c[:], h_t[:, kc, :],
                            start=(kc == 0), stop=(kc == KC - 1))
                        nc.tensor.matmul(
                            sq_ps[:], ones_c[:], sq_t[:],
                            start=(kc == 0), stop=(kc == KC - 1))
                    mean_r = sm5p.tile([1, NBLK], f32, name="mean5")
                    nc.vector.tensor_scalar_mul(mean_r[:], sum_ps[:], 1.0 / D)
                    ex2_r = sm5p.tile([1, NBLK], f32, name="ex25")
                    nc.vector.tensor_scalar_mul(ex2_r[:], sq_ps[:], 1.0 / D)
                    m2_r = sm5p.tile([1, NBLK], f32, name="m25")
                    nc.vector.tensor_mul(m2_r[:], mean_r[:], mean_r[:])
                    var_r = sm5p.tile([1, NBLK], f32, name="var5")
                    nc.vector.tensor_sub(var_r[:], ex2_r[:], m2_r[:])
                    sd_r = sm5p.tile([1, NBLK], f32, name="sd5")
                    nc.scalar.activation(sd_r[:], var_r[:], AF.Sqrt,
                                         bias=eps_c[0:1, :])
                    rstd_r = sm5p.tile([1, NBLK], f32, name="rstd5")
                    nc.vector.reciprocal(rstd_r[:], sd_r[:])
                    bcs = []
                    for nm, row in (("mean5", mean_r), ("rstd5", rstd_r)):
                        row16 = sm5p.tile([1, NBLK], bf16, name=f"r165_{nm}")
                        nc.vector.tensor_copy(row16[:], row[:])
                        bc = sm5p.tile([128, NBLK], bf16, name=f"bc5_{nm}")
                        bcast_rows(bc, row16, NBLK, psst5p, ones128)
                        bcs.append(bc)
                    mean_b, rstd_b = bcs
                    for kc in range(KC):
                        hm = sq5p.tile([128, NBLK], bf16, name="hm5")
                        nc.vector.tensor_sub(hm[:], h_t[:, kc, :], mean_b[:])
                        nc.vector.tensor_mul(
                            h2_t[:, kc, bsl], hm[:], rstd_b[:])
            xhres_pool.release()

            # ---------- phase 6: gated MLP + down proj + out --------------
            m_bnc = [dramp.tile([FFL, GB * NBLK], bf16, name=f"m_bnc{g}")
                     for g in range(NG)]
            m_ag = [dshp.tile([FF, GB * NBLK], bf16, name=f"m_ag{g}",
                              addr_space="Shared") for g in range(NG)]
            with (
                tc.tile_pool(name="wgu", bufs=1) as wgup,
                tc.tile_pool(name="wd", bufs=1) as wdp,
                tc.tile_pool(name="mloc", bufs=1) as mlocp,
                tc.tile_pool(name="gut", bufs=2) as gutp,
                tc.tile_pool(name="mchunk", bufs=3) as mchp,
                tc.tile_pool(name="outt", bufs=2) as outtp,
                tc.tile_pool(name="psG", bufs=2, space="PSUM") as psGp,
                tc.tile_pool(name="psU", bufs=2, space="PSUM") as psUp,
                tc.tile_pool(name="psDn", bufs=1, space="PSUM") as psDnp,
            ):
                wg_t = wgup.tile([128, KC, FFL], bf16)
                nc.sync.dma_start(wg_t[:], wg_e[:])
                wu_t = wgup.tile([128, KC, FFL], bf16)
                nc.sync.dma_start(wu_t[:], wu_e[:])
                wd_t = wdp.tile([128, FF // 128, J], bf16)
                nc.sync.dma_start(wd_t[:], wd_e[:])
                for b in range(NB):
                    bsl = slice(b * NBLK, (b + 1) * NBLK)
                    csl = slice((b % GB) * NBLK, (b % GB + 1) * NBLK)
                    m_loc = mlocp.tile([128, FFL // 128, NBLK], bf16,
                                       name="m_loc")
                    for mf in range(FFL // 128):
                        psg = psGp.tile([128, NBLK], f32, name="g_ps")
                        psu = psUp.tile([128, NBLK], f32, name="u_ps")
                        for kc in range(KC):
                            nc.tensor.matmul(
                                psg[:], wg_t[:, kc, mf * 128:(mf + 1) * 128],
                                h2_t[:, kc, bsl],
                                start=(kc == 0), stop=(kc == KC - 1))
                            nc.tensor.matmul(
                                psu[:], wu_t[:, kc, mf * 128:(mf + 1) * 128],
                                h2_t[:, kc, bsl],
                                start=(kc == 0), stop=(kc == KC - 1))
                        sg = gutp.tile([128, NBLK], bf16, name="sg")
                        nc.scalar.activation(
                            sg[:], psg[:], AF.Sigmoid,
                            bias=bg_t[:, mf:mf + 1])
                        silu = gutp.tile([128, NBLK], bf16, name="silu")
                        nc.vector.scalar_tensor_tensor(
                            out=silu[:], in0=psg[:],
                            scalar=bg_t[:, mf:mf + 1], in1=sg[:],
                            op0=Alu.add, op1=Alu.mult)
                        nc.vector.scalar_tensor_tensor(
                            out=m_loc[:, mf, :], in0=psu[:],
                            scalar=bu_t[:, mf:mf + 1], in1=silu[:],
                            op0=Alu.add, op1=Alu.mult)
                    nc.scalar.dma_start(
                        m_bnc[b // GB][:, csl].rearrange(
                            "(t p) s -> p t s", p=128),
                        m_loc[:])
                    if b % GB == GB - 1:
                        g = b // GB
                        nc.gpsimd.collective_compute(
                            "AllGather", mybir.AluOpType.bypass,
                            replica_groups=rg,
                            ins=[m_bnc[g].opt()], outs=[m_ag[g].opt()])
                # down projection per block (overlaps later groups' g/u)
                for b in range(NB):
                    bsl = slice(b * NBLK, (b + 1) * NBLK)
                    csl = slice((b % GB) * NBLK, (b % GB + 1) * NBLK)
                    dps = [psDnp.tile([128, NBLK], f32, name=f"d_ps{md}")
                           for md in range(MD)]
                    for fcg in range(FF // 512):
                        mch = mchp.tile([128, 4, NBLK], bf16, name="mch")
                        nc.sync.dma_start(
                            mch[:],
                            m_ag[b // GB][fcg * 512:(fcg + 1) * 512,
                                          csl].rearrange(
                                "(c p) s -> p c s", p=128))
                        for fci in range(4):
                            fc = fcg * 4 + fci
                            for md in range(MD):
                                nc.tensor.matmul(
                                    dps[md][:],
                                    wd_t[:, fc, md * 128:(md + 1) * 128],
                                    mch[:, fci, :],
                                    start=(fc == 0),
                                    stop=(fc == FF // 128 - 1))
                    for md in range(MD):
                        ot = outtp.tile([128, NBLK], f32, name="ot")
                        nc.vector.tensor_add(
                            ot[:], dps[md][:], res_t[:, md, bsl])
                        nc.scalar.dma_start(out_e[md, :, bsl], ot[:])
            h2_pool.release()
            res_pool.release()

    return nc


# ---------------------------------------------------------------------------
# Host side
# ---------------------------------------------------------------------------

def _chunkT(a):
    """[R, D] -> [128, D//128, R] view for lhsT/rhs chunk layout.

    Result[p, kc, r] = a[r, kc*128 + p].
    """
    R, Dd = a.shape
    return np.ascontiguousarray(
        a.reshape(R, Dd // 128, 128).transpose(2, 1, 0))


def prepare_inputs(hidden_states, memory, position_ids,
                   ln1_w, ln1_b, ln2_w, ln2_b,
                   Wq, Wk, Wv, Wo, Wg, Wu, Wd, S):
    """Build the 8 per-core in_maps (numpy host prep)."""
    f32 = np.float32
    hid = np.asarray(hidden_states, f32)[0]       # [S, D]
    mem = np.asarray(memory, f32)[0]
    pos = np.asarray(position_ids)[0].astype(np.float64)

    Wq1 = np.asarray(Wq, f32) * np.asarray(ln1_w, f32)[None, :]
    Wk1 = np.asarray(Wk, f32) * np.asarray(ln1_w, f32)[None, :]
    Wv1 = np.asarray(Wv, f32) * np.asarray(ln1_w, f32)[None, :]
    bq = np.asarray(Wq, f32) @ np.asarray(ln1_b, f32)
    bk = np.asarray(Wk, f32) @ np.asarray(ln1_b, f32)
    bv = np.asarray(Wv, f32) @ np.asarray(ln1_b, f32)
    Wg2 = np.asarray(Wg, f32) * np.asarray(ln2_w, f32)[None, :]
    Wu2 = np.asarray(Wu, f32) * np.asarray(ln2_w, f32)[None, :]
    bg = np.asarray(Wg, f32) @ np.asarray(ln2_b, f32)
    bu = np.asarray(Wu, f32) @ np.asarray(ln2_b, f32)
    Wo_ = np.asarray(Wo, f32)
    Wd_ = np.asarray(Wd, f32)

    # x^T chunk layouts (shared by all cores)
    xm = _chunkT(mem).astype(BF16)                # [128, KC, S]
    xh = _chunkT(hid).astype(BF16)

    # rope tables [128, 2S], row pattern period 16
    inv = BASE ** (-(np.arange(8, dtype=np.float64) * 2) / RD)
    t = pos[:, None] * inv[None, :]               # [S, 8]
    cos8 = np.cos(t).T                            # [8, S]
    sin8 = np.sin(t).T
    cos16 = np.concatenate([cos8, cos8], 0)       # [16, S]
    sin16 = np.concatenate([-sin8, sin8], 0)
    cosf = np.tile(np.concatenate([cos16, cos16], 1), (8, 1))  # [128, 2S]
    sinf = np.tile(np.concatenate([sin16, sin16], 1), (8, 1))
    cosf = cosf.astype(BF16)
    sinf = sinf.astype(BF16)

    # strict-causal masks for the 4 diagonal-band offsets
    ii = np.arange(128)[:, None]
    jj = np.arange(NBLK)[None, :]
    masks = np.stack(
        [(ii + 128 * o < jj) for o in range(4)], 1).astype(BF16)  # [128,4,512]

    in_maps = []
    for c in range(N_CORES):
        jsl = slice(c * J, (c + 1) * J)
        fsl = slice(c * FFL, (c + 1) * FFL)
        wq_c = Wq1[jsl]                            # [J, D]
        wk_c = Wk1[jsl]
        wv_c = Wv1[jsl]
        im = {
            "xm": xm, "xh": xh,
            "xh_res": np.ascontiguousarray(
                hid[:, c * J:(c + 1) * J].T.reshape(MD, 128, S)
                .transpose(1, 0, 2)).astype(f32),
            "wq": _chunkT(wq_c).astype(BF16),
            "wk": _chunkT(wk_c).astype(BF16),
            "wv": _chunkT(wv_c).astype(BF16),
            "wo": _chunkT(Wo_[jsl]).astype(BF16),
            "wg": _chunkT(Wg2[fsl]).astype(BF16),
            "wu": _chunkT(Wu2[fsl]).astype(BF16),
            "wd": _chunkT(Wd_[jsl]).astype(BF16),
            "wsq": np.ascontiguousarray(
                wq_c.sum(1).reshape(MD, 128).T).astype(f32),
            "wsk": np.ascontiguousarray(
                wk_c.sum(1).reshape(MD, 128).T).astype(f32),
            "wsvc": np.ascontiguousarray(
                wv_c.sum(1).reshape(MD, 128).T).astype(f32),
            "bq": np.ascontiguousarray(
                bq[jsl].reshape(MD, 128).T).astype(f32),
            "bk": np.ascontiguousarray(
                bk[jsl].reshape(MD, 128).T).astype(f32),
            "bvc": np.ascontiguousarray(
                bv[jsl].reshape(MD, 128).T).astype(f32),
            "wsv_row": wv_c.sum(1)[None, :].astype(f32),
            "bv_row": bv[jsl][None, :].astype(f32),
            "bg": np.ascontiguousarray(
                bg[fsl].reshape(FFL // 128, 128).T).astype(f32),
            "bu": np.ascontiguousarray(
                bu[fsl].reshape(FFL // 128, 128).T).astype(f32),
            "rope_cos": cosf, "rope_sinsg": sinf,
            "masks": masks,
        }
        in_maps.append(im)
    return in_maps


def assemble_output(results, S):
    outT = np.concatenate(
        [np.asarray(results[c]["out"]).reshape(J, S)
         for c in range(N_CORES)], 0)              # [D, S]
    return np.ascontiguousarray(outT.T).reshape(1, S, D).astype(np.float32)


_GRAPH_CACHE = {}


def get_graph(S):
    if S not in _GRAPH_CACHE:
        _GRAPH_CACHE[S] = build_graph(S)
    return _GRAPH_CACHE[S]


def kernel(hidden_states, memory, attention_mask, position_ids,
           ln1_w, ln1_b, ln2_w, ln2_b, Wq, Wk, Wv, Wo, Wg, Wu, Wd):
    from concourse.bass_utils import run_bass_kernel_spmd

    S = np.asarray(hidden_states).shape[1]
    in_maps = prepare_inputs(
        hidden_states, memory, position_ids, ln1_w, ln1_b, ln2_w, ln2_b,
        Wq, Wk, Wv, Wo, Wg, Wu, Wd, S)
    nc = get_graph(S)
    res = run_bass_kernel_spmd(nc, in_maps, core_ids=list(range(N_CORES)))
    return assemble_output(res.results, S)

